# revision 1
# baseline (speedup 1.0000x reference)
"""Trainium2 Bass kernel for the dense_cnn problem.

Computes out = (x + conv(x)) * t4 where
  conv = Conv2d(64->64, kernel (1,7), dilation (1,3), padding (0,9), no bias)
  t4[n,c,h,w] = sum_k p4w[k] * unfold3_dil2_h(x) rolled by (+1 h, -2 w)
             = roll_w(-2)[ p0*x[h-3] + p1*x[h-1] + p2*x[h+1] ]   (h taps via
               g=(h-1)%128; rows outside [0,128) contribute zero)

Sharding: pure data parallel, batch 32 -> 8 cores x 4 items. Each core
processes its 4 items as 2 "pairs": two batch items stacked on the 128
SBUF partitions (partition = 64*b + c).

Per pair, streamed over 32-row superblocks (descending h so edge rows for
h in {0,1,2} can read the tail rows captured into a small side tile):
  - PE: per 4-row PSUM block, identity matmul (residual, start=True) plus 7
    block-diagonal conv-tap matmuls on width-shifted views (float32r).
  - GPSIMD: U = sa*x[h+o0] + x[h+oj]      (two of the three h taps)
  - DVE:    V = sc*x[h+o2] + U            (third tap)
  - DVE:    out = (sm*psum) * V[w+2]      (final, PSUM read direct) plus a
            2-column fixup for the circular w roll.
"""

import sys

for _p in ("/opt/trn_rl_repo", "/opt/trn_rl_repo/concourse"):
    if _p not in sys.path:
        sys.path.insert(0, _p)

import numpy as np

N, C, H, W = 32, 64, 128, 128
N_CORES = 8
N_PER_CORE = N // N_CORES          # 4
PAIRS_PER_CORE = N_PER_CORE // 2   # 2
SB = 32                            # superblock rows
HALO_LO, HALO_HI = 3, 1            # x rows [s-3, s+33) needed per superblock
CHUNK_ROWS = SB + HALO_LO + HALO_HI  # 36
WP = W + 18                        # padded row stride for conv taps (9 each side)
TAP_OFFS = (-3, -1, 1)             # x-row offset of t4 tap k (bulk rows h>=3, h<=126)
CONV_D = tuple(3 * t - 9 for t in range(7))  # width offsets of the 7 conv taps

_CACHE = {}


def _special_terms(h):
    """(coeff_index, x_row) terms of t4 row h that fall inside [0, H)."""
    g = (h - 1) % H
    out = []
    for k in range(3):
        r = g + 2 * (k - 1)
        if 0 <= r < H:
            out.append((k, r))
    return out


def _build_bass(p):
    """Build the per-core Bass program. p = the 3 t4 tap coefficients."""
    import concourse.bass as bass
    import concourse.bacc as bacc
    import concourse.mybir as mybir
    import concourse.tile as tile

    dt = mybir.dt
    AL = mybir.AluOpType

    j = int(np.argmax(np.abs(p)))
    o0, o2 = [k for k in range(3) if k != j]
    sa = float(p[o0] / p[j])
    sc = float(p[o2] / p[j])
    sm = float(p[j])

    f32 = dt.float32
    f32r = dt.float32r

    nc = bacc.Bacc()
    x_d = nc.dram_tensor("x", [N_PER_CORE * C, H * W], f32r, kind="ExternalInput")
    w_d = nc.dram_tensor("wts", [128, 8 * 128], f32r, kind="ExternalInput")
    o_d = nc.dram_tensor("out", [N_PER_CORE * C, H * W], f32, kind="ExternalOutput")

    with tile.TileContext(nc) as tc:
        with (
            tc.tile_pool(name="wpool", bufs=1) as wpool,
            tc.tile_pool(name="chunk", bufs=3) as chp,
            tc.tile_pool(name="upool", bufs=2) as upool,
            tc.tile_pool(name="vpool", bufs=2) as vpool,
            tc.tile_pool(name="opool", bufs=3) as opool,
            tc.tile_pool(name="side", bufs=2) as sidep,
            tc.tile_pool(name="psum", bufs=8, space="PSUM") as psp,
        ):
            wt = wpool.tile([128, 8 * 128], f32r)
            nc.sync.dma_start(wt[:], w_d[:, :])

            for pair in range(PAIRS_PER_CORE):
                rows = slice(pair * 128, (pair + 1) * 128)
                side = sidep.tile([128, 4 * W], f32)  # x rows 124..127
                side3 = side[:].rearrange("p (h w) -> p h w", w=W)

                ch0_tile = None  # superblock s=0 chunk (x rows 0..32)
                for s in (96, 64, 32, 0):
                    lo = max(0, s - HALO_LO)
                    hi = min(H, s + SB + HALO_HI)
                    ch = chp.tile([128, CHUNK_ROWS * WP], f32r)
                    chp3 = ch[:].rearrange("p (h w) -> p h w", w=WP)
                    # zero the 9-col pads once per chunk (cheap, strided)
                    chpf = ch[:].bitcast(f32).rearrange("p (h w) -> p h w", w=WP)
                    nc.vector.memset(chpf[:, :, 0:9], 0.0)
                    nc.vector.memset(chpf[:, :, 9 + W : WP], 0.0)
                    # chunk row r  <->  x row (s - HALO_LO) + r
                    r0 = lo - (s - HALO_LO)
                    nc.sync.dma_start(
                        chp3[:, r0 : r0 + hi - lo, 9 : 9 + W],
                        x_d[rows, lo * W : hi * W],
                    )
                    ch3 = chp3[:, :, :]                                   # f32r, PE
                    chf = ch[:].bitcast(f32).rearrange("p (h w) -> p h w", w=WP)[:, :, 9 : 9 + W]
                    chr = lambda xr: xr - (s - HALO_LO)  # x row -> chunk row
                    if s == 96:
                        nc.gpsimd.tensor_copy(side3[:, :, :], chf[:, chr(124) : chr(128), :])
                    if s == 0:
                        ch0_tile = chf

                    # ---- t4 bulk: U on gpsimd, V on DVE ----
                    hlo = max(s, 3)
                    hhi = min(s + SB, 127)  # h=127 handled as a special
                    u = upool.tile([128, SB * W], f32)
                    v = vpool.tile([128, SB * W], f32)
                    u3 = u[:].rearrange("p (h w) -> p h w", w=W)
                    v3 = v[:].rearrange("p (h w) -> p h w", w=W)
                    bs = slice(hlo - s, hhi - s)  # tile-row range of the bulk

                    def cx(off):
                        return chf[:, hlo + off - (s - HALO_LO) : hhi + off - (s - HALO_LO), :]

                    # Pool has no STT: scale on ACT, add on GPSIMD (in-place)
                    nc.scalar.activation(
                        u3[:, bs, :], cx(TAP_OFFS[o0]),
                        mybir.ActivationFunctionType.Copy, scale=sa,
                    )
                    nc.gpsimd.tensor_add(u3[:, bs, :], u3[:, bs, :], cx(TAP_OFFS[j]))
                    nc.vector.scalar_tensor_tensor(
                        v3[:, bs, :], cx(TAP_OFFS[o2]), sc, u3[:, bs, :],
                        op0=AL.mult, op1=AL.add,
                    )

                    # ---- special t4 rows (unfold zero-pad x roll wrap) ----
                    specials = []
                    if s == 96:
                        specials = [127]
                    elif s == 0:
                        specials = [0, 1, 2]
                    for h in specials:
                        (ka, ra), (kb, rb) = _special_terms(h)
                        if abs(p[ka]) > abs(p[kb]):
                            (ka, ra), (kb, rb) = (kb, rb), (ka, ra)

                        def srcrow(r):
                            if s == 0 and r >= 124:
                                return side3[:, r - 124 : r - 123, :]
                            return chf[:, chr(r) : chr(r) + 1, :]

                        vrow = v3[:, h - s : h - s + 1, :]
                        nc.vector.scalar_tensor_tensor(
                            vrow, srcrow(ra), float(p[ka] / p[kb]), srcrow(rb),
                            op0=AL.mult, op1=AL.add,
                        )
                        nc.vector.tensor_scalar_mul(vrow, vrow, float(p[kb] / sm))

                    # ---- conv + residual on PE, final multiply on DVE ----
                    ot = opool.tile([128, SB * W], f32)
                    o3 = ot[:].rearrange("p (h w) -> p h w", w=W)
                    pss = [
                        psp.tile([128, 4 * W], f32, name="ps", tag="ps")
                        for _ in range(SB // 4)
                    ]
                    for jb in range(SB // 4):
                        hb = s + 4 * jb
                        ps = pss[jb]
                        ps3 = ps[:].rearrange("p (h w) -> p h w", w=W)
                        rh = slice(chr(hb), chr(hb) + 4)
                        # residual: out = I @ x (start=True initializes the bank)
                        nc.tensor.matmul(
                            ps3[:, :, :],
                            wt[:, 7 * 128 : 8 * 128],
                            ch3[:, rh, 9 : 9 + W],
                            start=True, stop=False,
                        )
                        for t in range(7):
                            d = CONV_D[t]
                            nc.tensor.matmul(
                                ps3[:, :, :],
                                wt[:, t * 128 : (t + 1) * 128],
                                ch3[:, rh, 9 + d : 9 + d + W],
                                start=False, stop=(t == 6),
                            )
                        tr = slice(4 * jb, 4 * jb + 4)
                        nc.vector.scalar_tensor_tensor(
                            o3[:, tr, 0 : W - 2], ps3[:, :, 0 : W - 2], sm,
                            v3[:, tr, 2:W], op0=AL.mult, op1=AL.mult,
                        )
                        nc.vector.scalar_tensor_tensor(
                            o3[:, tr, W - 2 : W], ps3[:, :, W - 2 : W], sm,
                            v3[:, tr, 0:2], op0=AL.mult, op1=AL.mult,
                        )
                    nc.sync.dma_start(o_d[rows, s * W : (s + SB) * W], ot[:])
    nc.compile()
    return nc


def kernel(x, W_conv, p4w):
    x = np.ascontiguousarray(x, dtype=np.float32)
    W_conv = np.asarray(W_conv, dtype=np.float32)
    p = np.asarray(p4w, dtype=np.float64).reshape(3)

    from concourse.bass_utils import run_bass_kernel_spmd

    key = tuple(np.round(p, 12))
    if key not in _CACHE:
        _CACHE[key] = _build_bass(p)
    nc = _CACHE[key]

    # weights: 7 block-diag conv taps + identity, lhsT layout (K=128, M=128)
    wts = np.zeros((128, 8 * 128), dtype=np.float32)
    wk = W_conv[:, :, 0, :]  # (O, I, T)
    for t in range(7):
        blk = wk[:, :, t].T  # (I, O) = lhsT block
        wts[0:64, t * 128 + 0 : t * 128 + 64] = blk
        wts[64:128, t * 128 + 64 : t * 128 + 128] = blk
    wts[:, 7 * 128 : 8 * 128] = np.eye(128, dtype=np.float32)

    xs = x.reshape(N_CORES, N_PER_CORE * C, H * W)
    in_maps = [{"x": xs[k], "wts": wts} for k in range(N_CORES)]
    res = run_bass_kernel_spmd(nc, in_maps, core_ids=list(range(N_CORES)))
    out = np.stack([res.results[k]["out"] for k in range(N_CORES)])
    return out.reshape(N, C, H, W)



# revision 13
# speedup vs baseline: 1.3718x; 1.3718x over previous
"""Trainium2 Bass kernel for the dense_cnn problem.

Computes out = (x + conv(x)) * t4 where
  conv = Conv2d(64->64, kernel (1,7), dilation (1,3), padding (0,9), no bias)
  t4[n,c,h,w] = roll_w(-2)[ p0*x[h-3] + p1*x[h-1] + p2*x[h+1] ]  (rows outside
                [0,128) contribute zero; h=0 wraps to rows 125/127)

Key optimizations over the naive formulation:
  - fp16 end-to-end: halves HBM traffic and the (slow) host<->device tunnel.
  - The residual (x +) is folded into the conv's center tap (offset 0):
    W'[:,:,3] += I.  No identity matmuls.
  - No zero-padded columns: conv taps with width offset d are issued as
    window-clipped matmuls; the full-width center tap runs first with
    start=True so every PSUM element is initialized.
  - The t4 h-taps use 3 zero-memset halo rows (x rows -3..-1 and 128) so the
    bulk 2-STT pipeline covers every h except h=0 (true circular wrap).
  - V is computed pre-rolled in w, so the final out = (sm*psum)*V is a single
    STT per 4-row PSUM block.
  - The PJRT runner is cached: one jit trace/compile per process, device-
    resident dummy output buffers, reshape views instead of concatenation.

Sharding: pure data parallel, batch 32 -> 8 cores x 4 items; each core
processes its items as 2 pairs of 2 stacked on the 128 SBUF partitions.
"""

import sys

for _p in ("/opt/trn_rl_repo", "/opt/trn_rl_repo/concourse"):
    if _p not in sys.path:
        sys.path.insert(0, _p)

import numpy as np

N, C, H, W = 32, 64, 128, 128
N_CORES = 8
N_PER_CORE = N // N_CORES          # 4
PAIRS_PER_CORE = N_PER_CORE // 2   # 2
SB = 32                            # superblock rows
HALO_LO, HALO_HI = 3, 1            # x rows [s-3, s+33) needed per superblock
CH_ROWS = SB + HALO_LO + HALO_HI   # 36
TAP_OFFS = (-3, -1, 1)             # x-row offset of t4 tap k (bulk rows)
CONV_D = tuple(3 * t - 9 for t in range(7))  # width offsets of the 7 conv taps
WP = W + 18                        # padded row stride for conv taps (9 each side)

_CACHE = {}
_RUNNER = {}


def _build_bass(p):
    """Build the per-core Bass program. p = the 3 t4 tap coefficients."""
    import concourse.bass as bass
    import concourse.bacc as bacc
    import concourse.mybir as mybir
    import concourse.tile as tile

    dt = mybir.dt
    AL = mybir.AluOpType

    j = int(np.argmax(np.abs(p)))
    o0, o2 = [k for k in range(3) if k != j]
    sa = float(p[o0] / p[j])
    sc = float(p[o2] / p[j])
    sm = float(p[j])

    f16 = dt.float16
    f32 = dt.float32

    nc = bacc.Bacc()
    x_d = nc.dram_tensor("x", [N_PER_CORE * C, H * W], f16, kind="ExternalInput")
    w_d = nc.dram_tensor("wts", [128, 7 * 128], f16, kind="ExternalInput")
    o_d = nc.dram_tensor("out", [N_PER_CORE * C, H * W], f16, kind="ExternalOutput")

    with tile.TileContext(nc) as tc:
        with (
            tc.tile_pool(name="wpool", bufs=1) as wpool,
            tc.tile_pool(name="chunk", bufs=3) as chp,
            tc.tile_pool(name="upool", bufs=2) as upool,
            tc.tile_pool(name="vpool", bufs=2) as vpool,
            tc.tile_pool(name="opool", bufs=3) as opool,
            tc.tile_pool(name="side", bufs=2) as sidep,
            tc.tile_pool(name="psum", bufs=8, space="PSUM") as psp,
        ):
            wt = wpool.tile([128, 7 * 128], f16)
            nc.sync.dma_start(wt[:], w_d[:, :])

            for pair in range(PAIRS_PER_CORE):
                rows = slice(pair * 128, (pair + 1) * 128)
                side = sidep.tile([128, 2 * W], f16)  # x rows 125, 127
                side3 = side[:].rearrange("p (h w) -> p h w", w=W)

                for s in (96, 64, 32, 0):
                    lo = max(0, s - HALO_LO)
                    hi = min(H, s + SB + HALO_HI)
                    ch = chp.tile([128, CH_ROWS * W], f16)
                    chf = ch[:].rearrange("p (h w) -> p h w", w=W)
                    chw = ch[:].rearrange("p (h w) -> p w h", w=W)  # transposed view
                    chr_ = lambda xr: xr - (s - HALO_LO)  # x row -> chunk row
                    # chunk row r  <->  x row (s - HALO_LO) + r
                    r0 = lo - (s - HALO_LO)
                    if s == 0:
                        nc.vector.memset(chf[:, 0:3, :], 0.0)  # x rows -3..-1
                    if s == 96:
                        nc.vector.memset(chf[:, chr_(128) : chr_(128) + 1, :], 0.0)
                    nc.sync.dma_start(
                        chf[:, r0 : r0 + hi - lo, :],
                        x_d[rows, lo * W : hi * W],
                    )
                    if s == 96:
                        nc.gpsimd.tensor_copy(
                            side3[:, 0:1, :], chf[:, chr_(125) : chr_(126), :]
                        )
                        nc.gpsimd.tensor_copy(
                            side3[:, 1:2, :], chf[:, chr_(127) : chr_(128), :]
                        )

                    # ---- t4 bulk: U then pre-w-rolled V, both on DVE ----
                    # U[i] = sa*x[s+i-3] + x[s+i-1]; V[i,w] = sc*x[s+i+1,(w+2)%W] + U[i,(w+2)%W]
                    u = upool.tile([128, SB * W], f16)
                    v = vpool.tile([128, SB * W], f16)
                    u3 = u[:].rearrange("p (h w) -> p h w", w=W)
                    v3 = v[:].rearrange("p (h w) -> p h w", w=W)
                    nc.scalar.activation(
                        u3[:, :, :], chf[:, 0:SB, :],
                        mybir.ActivationFunctionType.Copy, scale=sa,
                    )
                    nc.gpsimd.tensor_add(u3[:, :, :], u3[:, :, :], chf[:, 2 : SB + 2, :])
                    nc.vector.scalar_tensor_tensor(
                        v3[:, :, 0 : W - 2], chf[:, 4 : SB + 4, 2:W], sc,
                        u3[:, :, 2:W], op0=AL.mult, op1=AL.add,
                    )
                    nc.vector.scalar_tensor_tensor(
                        v3[:, :, W - 2 : W], chf[:, 4 : SB + 4, 0:2], sc,
                        u3[:, :, 0:2], op0=AL.mult, op1=AL.add,
                    )

                    # ---- special t4 row h=0: p0*x[125] + p1*x[127] (wrap) ----
                    if s == 0:
                        ka, kb = (0, 1) if abs(p[0]) <= abs(p[1]) else (1, 0)
                        A = side3[:, ka : ka + 1, :]  # row 125 is index 0
                        B = side3[:, kb : kb + 1, :]
                        r = float(p[ka] / p[kb])
                        vrow = v3[:, 0:1, :]
                        nc.vector.scalar_tensor_tensor(
                            vrow[:, :, 0 : W - 2], A[:, :, 2:W], r, B[:, :, 2:W],
                            op0=AL.mult, op1=AL.add,
                        )
                        nc.vector.scalar_tensor_tensor(
                            vrow[:, :, W - 2 : W], A[:, :, 0:2], r, B[:, :, 0:2],
                            op0=AL.mult, op1=AL.add,
                        )
                        nc.vector.tensor_scalar_mul(vrow, vrow, float(p[kb] / sm))

                    # ---- conv (+ folded residual) on PE, final mult on DVE ----
                    # PSUM is w-major (elem (w,h): offset w*4+h) so clipped
                    # tap windows stay contiguous 2D for the matmul out AP.
                    ot = opool.tile([128, SB * W], f16)
                    o3 = ot[:].rearrange("p (h w) -> p h w", w=W)
                    for jb in range(SB // 4):
                        hb = s + 4 * jb
                        ps = psp.tile([128, 4 * W], f32, name="ps", tag="ps")
                        ps_wm = ps[:].rearrange("p (w h) -> p w h", h=4)
                        ps_hm = ps[:].rearrange("p (w h) -> p h w", h=4)
                        rh = slice(chr_(hb), chr_(hb) + 4)
                        # center tap (d=0, includes +I residual) starts PSUM
                        nc.tensor.matmul(
                            ps_wm[:, :, :],
                            wt[:, 3 * 128 : 4 * 128],
                            chw[:, :, rh],
                            start=True, stop=False,
                        )
                        for t in (0, 1, 2, 4, 5, 6):
                            d = CONV_D[t]
                            w0 = max(0, -d)
                            w1 = W - max(0, d)
                            nc.tensor.matmul(
                                ps_wm[:, w0:w1, :],
                                wt[:, t * 128 : (t + 1) * 128],
                                chw[:, w0 + d : w1 + d, rh],
                                start=False, stop=(t == 6),
                            )
                        tr = slice(4 * jb, 4 * jb + 4)
                        nc.vector.scalar_tensor_tensor(
                            o3[:, tr, :], ps_hm[:, :, :], sm, v3[:, tr, :],
                            op0=AL.mult, op1=AL.mult,
                        )
                    nc.sync.dma_start(o_d[rows, s * W : (s + SB) * W], ot[:])
    nc.compile()
    return nc


def _make_wts(W_conv):
    """(128, 7*128) fp16 lhsT weights: block-diag conv taps, +I on center."""
    wts = np.zeros((128, 7 * 128), dtype=np.float32)
    wk = np.asarray(W_conv, dtype=np.float32)[:, :, 0, :]  # (O, I, T)
    for t in range(7):
        blk = wk[:, :, t].T  # (I, O) = lhsT block
        wts[0:64, t * 128 + 0 : t * 128 + 64] = blk
        wts[64:128, t * 128 + 64 : t * 128 + 128] = blk
    wts[:, 3 * 128 : 4 * 128] += np.eye(128, dtype=np.float32)
    return wts.astype(np.float16)


def _core_inputs(x, W_conv, core):
    """Per-core input map (used by the CoreSim driver)."""
    xs = np.ascontiguousarray(x, dtype=np.float16).reshape(
        N_CORES, N_PER_CORE * C, H * W
    )
    return {"x": xs[core], "wts": _make_wts(W_conv)}


def _build_runner(nc):
    """Cached PJRT runner: one jit trace, device-resident dummy out buffers."""
    import jax
    from jax.sharding import Mesh, PartitionSpec, NamedSharding

    try:
        from jax.shard_map import shard_map
    except ImportError:
        from jax.experimental.shard_map import shard_map

    from concourse import bass2jax, mybir

    bass2jax.install_neuronx_cc_hook()

    in_names, out_names, out_avals, zero_outs = [], [], [], []
    for alloc in nc.m.functions[0].allocations:
        if not isinstance(alloc, mybir.MemoryLocationSet):
            continue
        name = alloc.memorylocations[0].name
        if alloc.kind == "ExternalInput":
            in_names.append(name)
        elif alloc.kind == "ExternalOutput":
            out_names.append(name)
            shape = tuple(alloc.tensor_shape)
            dtype = mybir.dt.np(alloc.dtype)
            out_avals.append(jax.core.ShapedArray(shape, dtype))
            zero_outs.append(np.zeros((N_CORES * shape[0], *shape[1:]), dtype))
    n_params = len(in_names)
    in_names = in_names + out_names

    def _body(*args):
        outs = bass2jax._bass_exec_p.bind(
            *args,
            out_avals=tuple(out_avals),
            in_names=tuple(in_names),
            out_names=tuple(out_names),
            lowering_input_output_aliases=(),
            sim_require_finite=True,
            sim_require_nnan=True,
            nc=nc,
        )
        return tuple(outs)

    devices = jax.devices()[:N_CORES]
    mesh = Mesh(np.asarray(devices), ("core",))
    spec = PartitionSpec("core")
    sharded = jax.jit(
        shard_map(
            _body,
            mesh=mesh,
            in_specs=(spec,) * (n_params + len(out_names)),
            out_specs=(spec,) * len(out_names),
            check_rep=False,
        ),
        keep_unused=True,
    )
    sharding = NamedSharding(mesh, spec)
    # The zero "output" operands are unused by the custom call (our kernel
    # writes every output element); keep them device-resident across calls.
    zeros_dev = [jax.device_put(z, sharding) for z in zero_outs]
    return sharded, zeros_dev


def kernel(x, W_conv, p4w):
    p = np.asarray(p4w, dtype=np.float64).reshape(3)
    key = tuple(np.round(p, 12))
    if key not in _CACHE:
        _CACHE[key] = _build_bass(p)
    nc = _CACHE[key]

    xg = np.ascontiguousarray(x, dtype=np.float16).reshape(
        N_CORES * N_PER_CORE * C, H * W
    )
    wg = np.tile(_make_wts(W_conv), (N_CORES, 1))

    try:
        if key not in _RUNNER:
            _RUNNER[key] = _build_runner(nc)
        sharded, zeros_dev = _RUNNER[key]
        (out_g,) = sharded(xg, wg, *zeros_dev)
        out = np.asarray(out_g)
    except Exception:
        from concourse.bass_utils import run_bass_kernel_spmd

        xs = xg.reshape(N_CORES, N_PER_CORE * C, H * W)
        wts = _make_wts(W_conv)
        in_maps = [{"x": xs[k], "wts": wts} for k in range(N_CORES)]
        res = run_bass_kernel_spmd(nc, in_maps, core_ids=list(range(N_CORES)))
        out = np.stack([res.results[k]["out"] for k in range(N_CORES)])

    return out.astype(np.float32).reshape(N, C, H, W)


# revision 32
# speedup vs baseline: 1.7231x; 1.2561x over previous
"""Trainium2 Bass kernel for the dense_cnn problem.

Computes out = (x + conv(x)) * t4 where
  conv = Conv2d(64->64, kernel (1,7), dilation (1,3), padding (0,9), no bias)
  t4[n,c,h,w] = roll_w(-2)[ p0*x[h-3] + p1*x[h-1] + p2*x[h+1] ]  (rows outside
                [0,128) contribute zero; h=0 wraps to rows 125/127)

Key optimizations over the naive formulation:
  - fp16 end-to-end: halves HBM traffic and the (slow) host<->device tunnel.
  - The residual (x +) is folded into the conv's center tap (offset 0):
    W'[:,:,3] += I.  No identity matmuls.
  - No zero-padded columns: conv taps with width offset d are issued as
    window-clipped matmuls; the full-width center tap runs first with
    start=True so every PSUM element is initialized.
  - The t4 h-taps use 3 zero-memset halo rows (x rows -3..-1 and 128) so the
    bulk 2-STT pipeline covers every h except h=0 (true circular wrap).
  - V is computed pre-rolled in w, so the final out = (sm*psum)*V is a single
    STT per 4-row PSUM block.
  - The PJRT runner is cached: one jit trace/compile per process, device-
    resident dummy output buffers, reshape views instead of concatenation.

Sharding: pure data parallel, batch 32 -> 8 cores x 4 items; each core
processes its items as 2 pairs of 2 stacked on the 128 SBUF partitions.
"""

import sys

for _p in ("/opt/trn_rl_repo", "/opt/trn_rl_repo/concourse"):
    if _p not in sys.path:
        sys.path.insert(0, _p)

import numpy as np

N, C, H, W = 32, 64, 128, 128
N_CORES = 8
N_PER_CORE = N // N_CORES          # 4
PAIRS_PER_CORE = N_PER_CORE // 2   # 2
SB = 32                            # superblock rows
HALO_LO, HALO_HI = 3, 1            # x rows [s-3, s+33) needed per superblock
CH_ROWS = SB + HALO_LO + HALO_HI   # 36
TAP_OFFS = (-3, -1, 1)             # x-row offset of t4 tap k (bulk rows)
CONV_D = tuple(3 * t - 9 for t in range(7))  # width offsets of the 7 conv taps
WP = W + 18                        # padded row stride for conv taps (9 each side)

_CACHE = {}
_RUNNER = {}


def _build_bass(p):
    """Build the per-core Bass program. p = the 3 t4 tap coefficients."""
    import concourse.bass as bass
    import concourse.bacc as bacc
    import concourse.mybir as mybir
    import concourse.tile as tile

    dt = mybir.dt
    AL = mybir.AluOpType

    j = int(np.argmax(np.abs(p)))
    o0, o2 = [k for k in range(3) if k != j]
    sa = float(p[o0] / p[j])
    sc = float(p[o2] / p[j])
    sm = float(p[j])

    f16 = dt.float16
    f32 = dt.float32
    f8 = dt.float8e4

    nc = bacc.Bacc()
    x_d = nc.dram_tensor("x", [N_PER_CORE * C, H * W], f16, kind="ExternalInput")
    w_d = nc.dram_tensor("wts8", [128, 8 * 2 * 128], f8, kind="ExternalInput")
    o_d = nc.dram_tensor("out", [N_PER_CORE * C, H * W], f16, kind="ExternalOutput")

    with tile.TileContext(nc) as tc:
        with (
            tc.tile_pool(name="wpool", bufs=1) as wpool,
            tc.tile_pool(name="chunk", bufs=4) as chp,
            tc.tile_pool(name="c8pool", bufs=4) as c8p,
            tc.tile_pool(name="upool", bufs=4) as upool,
            tc.tile_pool(name="vpool", bufs=4) as vpool,
            tc.tile_pool(name="tpool", bufs=2) as tpool,
            tc.tile_pool(name="opool", bufs=3) as opool,
            tc.tile_pool(name="side", bufs=2) as sidep,
            tc.tile_pool(name="psum", bufs=8, space="PSUM") as psp,
        ):
            wt8 = wpool.tile([128, 8 * 2 * 128], f8)
            nc.sync.dma_start(wt8[:], w_d[:, :])
            wt8r = wt8[:].rearrange("p (t pl m) -> p t pl m", pl=2, m=128)

            sides = {}

            def prep(pair, s):
                """Load + build fp8 planes + t4 V for one superblock."""
                rows = slice(pair * 128, (pair + 1) * 128)
                lo = max(0, s - HALO_LO)
                hi = min(H, s + SB + HALO_HI)
                ch = chp.tile([128, CH_ROWS * W], f16)
                chf = ch[:].rearrange("p (h w) -> p h w", w=W)
                chr_ = lambda xr: xr - (s - HALO_LO)  # x row -> chunk row
                r0 = lo - (s - HALO_LO)
                if s == 0:
                    nc.vector.memset(chf[:, 0:3, :], 0.0)  # x rows -3..-1
                if s == 96:
                    nc.vector.memset(chf[:, chr_(128) : chr_(128) + 1, :], 0.0)
                mid = lo + 20 - r0  # split the load so early rows land sooner
                nc.sync.dma_start(
                    chf[:, r0:20, :], x_d[rows, lo * W : mid * W]
                )
                nc.sync.dma_start(
                    chf[:, 20 : r0 + hi - lo, :], x_d[rows, mid * W : hi * W]
                )
                if s == 96:
                    side = sidep.tile([128, 2 * W], f16)  # x rows 125, 127
                    side3 = side[:].rearrange("p (h w) -> p h w", w=W)
                    nc.gpsimd.tensor_copy(
                        side3[:, 0:1, :], chf[:, chr_(125) : chr_(126), :]
                    )
                    nc.gpsimd.tensor_copy(
                        side3[:, 1:2, :], chf[:, chr_(127) : chr_(128), :]
                    )
                    sides[pair] = side3

                # fp8 hi/lo planes, per-block w-major (idx = jb*1024+pl*512+w*4+h)
                ch8 = c8p.tile([128, 8 * 2 * 512], f8)
                ch8w = ch8[:].rearrange("p (jb pl w h) -> p pl jb w h", pl=2, w=W, h=4)
                ch8b = ch8[:].rearrange("p (jb pl f) -> p jb pl f", pl=2, f=512)
                # conv rows [s, s+32) = chunk rows [3, 35), as (jb, w, h)
                csrc = chf[:, HALO_LO : HALO_LO + SB, :].rearrange(
                    "p (jb h) w -> p jb w h", h=4
                )
                for q in range(4):
                    hf = slice(2 * q, 2 * q + 2)
                    nc.scalar.activation(
                        ch8w[:, 0, hf], csrc[:, hf],
                        mybir.ActivationFunctionType.Copy,
                    )
                    nc.gpsimd.tensor_sub(ch8w[:, 1, hf], csrc[:, hf], ch8w[:, 0, hf])

                # t4 bulk: U = sa*x[h-3] + x[h-1] (DVE scale + Pool add);
                # V[i,w] = sc*x[h+1,(w+2)%W] + U[i,(w+2)%W] (pre-rolled, DVE)
                u = upool.tile([128, SB * W], f16)
                v = vpool.tile([128, SB * W], f16)
                u3 = u[:].rearrange("p (h w) -> p h w", w=W)
                v3 = v[:].rearrange("p (h w) -> p h w", w=W)
                nc.vector.tensor_scalar_mul(u3[:, :, :], chf[:, 0:SB, :], sa)
                nc.gpsimd.tensor_add(u3[:, :, :], u3[:, :, :], chf[:, 2 : SB + 2, :])
                nc.vector.scalar_tensor_tensor(
                    v3[:, :, 0 : W - 2], chf[:, 4 : SB + 4, 2:W], sc,
                    u3[:, :, 2:W], op0=AL.mult, op1=AL.add,
                )
                nc.vector.scalar_tensor_tensor(
                    v3[:, :, W - 2 : W], chf[:, 4 : SB + 4, 0:2], sc,
                    u3[:, :, 0:2], op0=AL.mult, op1=AL.add,
                )

                # special t4 row h=0: p0*x[125] + p1*x[127] (circular wrap)
                if s == 0:
                    side3 = sides[pair]
                    ka, kb = (0, 1) if abs(p[0]) <= abs(p[1]) else (1, 0)
                    A = side3[:, ka : ka + 1, :]  # row 125 is index 0
                    B = side3[:, kb : kb + 1, :]
                    r = float(p[ka] / p[kb])
                    vrow = v3[:, 0:1, :]
                    nc.vector.scalar_tensor_tensor(
                        vrow[:, :, 0 : W - 2], A[:, :, 2:W], r, B[:, :, 2:W],
                        op0=AL.mult, op1=AL.add,
                    )
                    nc.vector.scalar_tensor_tensor(
                        vrow[:, :, W - 2 : W], A[:, :, 0:2], r, B[:, :, 0:2],
                        op0=AL.mult, op1=AL.add,
                    )
                    nc.vector.tensor_scalar_mul(vrow, vrow, float(p[kb] / sm))
                return pair, s, ch8b, v3

            def compute(state):
                """Conv + residual on PE (fp8 DoubleRow); finals; store."""
                pair, s, ch8b, v3 = state
                rows = slice(pair * 128, (pair + 1) * 128)
                # PSUM is w-major (elem (w,h): offset w*4+h) so clipped tap
                # windows stay contiguous 2D for the matmul out AP.
                ot = opool.tile([128, SB * W], f16)
                o3 = ot[:].rearrange("p (h w) -> p h w", w=W)
                tmp = tpool.tile([128, 4 * 4 * W], f16)
                tmp3 = tmp[:].rearrange("p (b h w) -> p b h w", h=4, w=W)
                for jb in range(SB // 4):
                    ps = psp.tile([128, 4 * W], f32, name="ps", tag="ps")
                    ps_wm = ps[:].rearrange("p (w h) -> p w h", h=4)
                    ps_hm = ps[:].rearrange("p (w h) -> p h w", h=4)
                    # identity tap (residual, full width) starts PSUM
                    nc.tensor.matmul(
                        ps_wm[:, :, :],
                        wt8r[:, 7],
                        ch8b[:, jb, :, :],
                        start=True, stop=False,
                        perf_mode=mybir.MatmulPerfMode.DoubleRow,
                    )
                    for t in (3, 0, 1, 2, 4, 5, 6):
                        d = CONV_D[t]
                        w0 = max(0, -d)
                        w1 = W - max(0, d)
                        nc.tensor.matmul(
                            ps_wm[:, w0:w1, :],
                            wt8r[:, t],
                            ch8b[:, jb, :, 4 * (w0 + d) : 4 * (w1 + d)],
                            start=False, stop=(t == 6),
                            perf_mode=mybir.MatmulPerfMode.DoubleRow,
                        )
                    tr = slice(4 * jb, 4 * jb + 4)
                    if jb < 4:
                        nc.scalar.activation(
                            tmp3[:, jb], ps_hm[:, :, :],
                            mybir.ActivationFunctionType.Copy, scale=sm,
                        )
                        nc.gpsimd.tensor_mul(o3[:, tr, :], tmp3[:, jb], v3[:, tr, :])
                    else:
                        nc.vector.scalar_tensor_tensor(
                            o3[:, tr, :], ps_hm[:, :, :], sm, v3[:, tr, :],
                            op0=AL.mult, op1=AL.mult,
                        )
                hw = SB // 2 * W
                nc.sync.dma_start(o_d[rows, s * W : s * W + hw], ot[:, 0:hw])
                nc.sync.dma_start(o_d[rows, s * W + hw : (s + SB) * W], ot[:, hw:])

            # software-pipelined emission: prep leads compute by two superblocks
            work = [(pair, s) for pair in range(PAIRS_PER_CORE) for s in (96, 64, 32, 0)]
            from collections import deque

            pend = deque()
            for pair, s in work:
                pend.append(prep(pair, s))
                if len(pend) > 2:
                    compute(pend.popleft())
            while pend:
                compute(pend.popleft())
    nc.compile()
    return nc


def _make_wts8(W_conv):
    """(128, 8*2*128) fp8e4m3 DoubleRow lhsT: per tap 2 identical block-diag
    planes (one for x_hi, one for x_lo); tap 7 = identity (the residual)."""
    import ml_dtypes

    wts = np.zeros((128, 8, 2, 128), dtype=np.float32)
    wk = np.asarray(W_conv, dtype=np.float32)[:, :, 0, :]  # (O, I, T)
    for t in range(7):
        blk = wk[:, :, t].T  # (I, O) = lhsT block
        for pl in range(2):
            wts[0:64, t, pl, 0:64] = blk
            wts[64:128, t, pl, 64:128] = blk
    wts[:, 7, 0] = np.eye(128, dtype=np.float32)
    wts[:, 7, 1] = np.eye(128, dtype=np.float32)
    return wts.reshape(128, 8 * 2 * 128).astype(ml_dtypes.float8_e4m3fn)


def _core_inputs(x, W_conv, core):
    """Per-core input map (used by the CoreSim driver)."""
    xs = np.ascontiguousarray(x, dtype=np.float16).reshape(
        N_CORES, N_PER_CORE * C, H * W
    )
    return {"x": xs[core], "wts8": _make_wts8(W_conv)}


def _build_runner(nc):
    """Cached PJRT runner: one jit trace, device-resident dummy out buffers."""
    import jax
    from jax.sharding import Mesh, PartitionSpec, NamedSharding

    try:
        from jax.shard_map import shard_map
    except ImportError:
        from jax.experimental.shard_map import shard_map

    from concourse import bass2jax, mybir

    bass2jax.install_neuronx_cc_hook()

    part_name = nc.partition_id_tensor.name if nc.partition_id_tensor else None
    in_names, out_names, out_avals, zero_outs = [], [], [], []
    for alloc in nc.m.functions[0].allocations:
        if not isinstance(alloc, mybir.MemoryLocationSet):
            continue
        name = alloc.memorylocations[0].name
        if alloc.kind == "ExternalInput":
            if name != part_name:
                in_names.append(name)
        elif alloc.kind == "ExternalOutput":
            out_names.append(name)
            shape = tuple(alloc.tensor_shape)
            dtype = mybir.dt.np(alloc.dtype)
            out_avals.append(jax.core.ShapedArray(shape, dtype))
            zero_outs.append(np.zeros((N_CORES * shape[0], *shape[1:]), dtype))
    n_params = len(in_names)
    in_names = in_names + out_names
    if part_name is not None:
        in_names.append(part_name)

    def _body(*args):
        operands = list(args)
        if part_name is not None:
            operands.append(bass2jax.partition_id_tensor())
        outs = bass2jax._bass_exec_p.bind(
            *operands,
            out_avals=tuple(out_avals),
            in_names=tuple(in_names),
            out_names=tuple(out_names),
            lowering_input_output_aliases=(),
            sim_require_finite=True,
            sim_require_nnan=True,
            nc=nc,
        )
        return tuple(outs)

    devices = jax.devices()[:N_CORES]
    mesh = Mesh(np.asarray(devices), ("core",))
    spec = PartitionSpec("core")
    sharded = jax.jit(
        shard_map(
            _body,
            mesh=mesh,
            in_specs=(spec,) * (n_params + len(out_names)),
            out_specs=(spec,) * len(out_names),
            check_rep=False,
        ),
        keep_unused=True,
    )
    sharding = NamedSharding(mesh, spec)
    # The zero "output" operands are unused by the custom call (our kernel
    # writes every output element); keep them device-resident across calls.
    zeros_dev = [jax.device_put(z, sharding) for z in zero_outs]
    return sharded, zeros_dev


def _cast(src, dtype, out=None):
    """Multithreaded dtype cast (the arrays are ~35-70MB; numpy is 1-thread)."""
    from concurrent.futures import ThreadPoolExecutor

    flat = np.ascontiguousarray(src).reshape(-1)
    res = np.empty(flat.shape, dtype) if out is None else out.reshape(-1)
    nt = 8
    step = (flat.size + nt - 1) // nt

    def go(i):
        sl = slice(i * step, min((i + 1) * step, flat.size))
        res[sl] = flat[sl]

    with ThreadPoolExecutor(nt) as pool:
        list(pool.map(go, range(nt)))
    return res


def kernel(x, W_conv, p4w):
    p = np.asarray(p4w, dtype=np.float64).reshape(3)
    key = tuple(np.round(p, 12))
    if key not in _CACHE:
        _CACHE[key] = _build_bass(p)
    nc = _CACHE[key]

    xg = _cast(x, np.float16).reshape(N_CORES * N_PER_CORE * C, H * W)
    wg = np.tile(_make_wts8(W_conv), (N_CORES, 1))

    try:
        if key not in _RUNNER:
            _RUNNER[key] = _build_runner(nc)
        sharded, zeros_dev = _RUNNER[key]
        (out_g,) = sharded(xg, wg, *zeros_dev)
        out = np.asarray(out_g)
    except Exception:
        from concourse.bass_utils import run_bass_kernel_spmd

        xs = xg.reshape(N_CORES, N_PER_CORE * C, H * W)
        wts = _make_wts8(W_conv)
        in_maps = [{"x": xs[k], "wts8": wts} for k in range(N_CORES)]
        res = run_bass_kernel_spmd(nc, in_maps, core_ids=list(range(N_CORES)))
        out = np.stack([res.results[k]["out"] for k in range(N_CORES)])

    return _cast(out, np.float32).reshape(N, C, H, W)
